# revision 15
# baseline (speedup 1.0000x reference)
"""Multi-head self-attention Trainium2 kernel (8 NeuronCores, SPMD).

Problem: B=2, S=2048, D=1024, H=16, Dk=64; torch-style Linear projections
(x @ W.T + b), custom softmax: p = exp(scores/8), attn = p / (sum(p) + 1e-8).

Sharding: 32 (batch, head) pairs over 8 cores -> core c handles batch c//4,
heads [4*(c%4), 4*(c%4)+4). Each core projects only its 256 features of
q/k/v; attention is embarrassingly parallel over (b, h).

v2 changes over the 232us baseline (trace-driven):
  - The kernel was PE+ACT co-bound (tensor_engine_active 202us, ACT exp
    143us).  PSUM write bus is 1 fp32/cycle/partition: the row-grouped
    score pair is bus-capped at ~491ns regardless of dtype, so scores
    stay fp32r.  The AV path and v-projection move to bf16 (1 cyc/col
    vs fp32r's 1.33): pT (exp output) is bf16, v_ext is bf16 with bv
    pre-folded (exact: sum_t p*bv = denom*bv), wv is bf16.
  - exp is split between ACT (exact, 1.12us/tile) and DVE via a
    Schraudolph bit-trick (i16 = round(score*A + B) bitcast to bf16,
    one tensor_scalar, ~1.24us/tile, +-3.5% on p; end-to-end sim err
    1.0e-2 at 50% offload vs the 2e-2 gate).  Pattern: i%8 in {2,5,7}.
    The two exp engines also overlap adjacent pipeline steps.
  - ACT takes the qk bias-adds (activation Identity, bias is in the
    exp table set: no reload); its DMA issues move to sync/gpsimd/DVE
    queues.  DVE keeps v-fold, ctx copies, reciprocals, normalize.
  - transpose/normalize: 4 transposes of one cs go into one PSUM tile
    so a single strided reciprocal covers all 4 denominators.

Scheduling: unchanged 128-step software pipeline with deficit-paced
filler (qT[0] chunks 1-3, pair-1 projections, transpose/normalize).

Output per core: [2048, 256] fp32 -> host concatenates features per batch.
"""

import sys

sys.path.insert(0, "/opt/trn_rl_repo")

from collections import deque
from contextlib import ExitStack

import numpy as np
import ml_dtypes

import concourse.bass as bass
import concourse.tile as tile
from concourse import bacc, mybir
from concourse.bass_utils import run_bass_kernel_spmd
from concourse.masks import make_identity

F32 = mybir.dt.float32
F32R = mybir.dt.float32r
BF16 = mybir.dt.bfloat16
I16 = mybir.dt.int16

S = 2048  # sequence length
D = 1024  # d_model
J = 256  # features per core (4 heads x 64)
NKT = 8  # k-tiles of the d_model contraction
NSC = 4  # s-chunks of 512
NTT = 16  # t-tiles of 128
N_CORES = 8

# Schraudolph exp2 constants in the bf16-bit domain (exp(x/8) ~ bf16 bits of
# round(x * SCH_A + SCH_B)); c=6 centers the +-3% relative error.
SCH_A = float(128 * np.log2(np.e) / 8)
SCH_B = float(127 * 128 - 6)

# which of the 128 pipeline steps compute exp on the DVE instead of ACT
# (chosen away from block boundaries so a DVE exp never queues behind the
# block-end copy/normalize burst)
DVE_EXP = frozenset(i for i in range(128) if i % 8 in (2, 5, 7))

_cached_nc = None
last_result = None  # BassKernelResults of the most recent run (for test.py)


def _round_fp32r(x: np.ndarray) -> np.ndarray:
    """Round fp32 to fp32r (keep 11 mantissa bits, round to nearest even)."""
    u = np.ascontiguousarray(x, dtype=np.float32).view(np.uint32)
    r = (u.astype(np.uint64) + 0x7FF + ((u >> 12) & 1)) & 0xFFFFF000
    return r.astype(np.uint32).view(np.float32)


def _build():
    nc = bacc.Bacc(None, target_bir_lowering=False)

    qt = nc.dram_tensor("qt", [D, S], BF16, kind="ExternalInput")
    wq = nc.dram_tensor("wq", [D, J], BF16, kind="ExternalInput")
    wk = nc.dram_tensor("wk", [D, J], BF16, kind="ExternalInput")
    wv = nc.dram_tensor("wv", [D, J], BF16, kind="ExternalInput")
    bq = nc.dram_tensor("bq", [J], F32, kind="ExternalInput")
    bk = nc.dram_tensor("bk", [J], F32, kind="ExternalInput")
    bv = nc.dram_tensor("bv", [J], F32, kind="ExternalInput")
    out = nc.dram_tensor("out", [S, J], F32, kind="ExternalOutput")

    IDEN = mybir.ActivationFunctionType.Identity
    EXP = mybir.ActivationFunctionType.Exp

    with tile.TileContext(nc) as tc, ExitStack() as ctx:
        wts = ctx.enter_context(tc.tile_pool(name="wts", bufs=1))
        qkp = ctx.enter_context(tc.tile_pool(name="qkp", bufs=1))
        vxp = ctx.enter_context(tc.tile_pool(name="vxp", bufs=1))
        bp = ctx.enter_context(tc.tile_pool(name="bp", bufs=1))
        cxp = ctx.enter_context(tc.tile_pool(name="cxp", bufs=6))
        pTp = ctx.enter_context(tc.tile_pool(name="pTp", bufs=4))
        outp = ctx.enter_context(tc.tile_pool(name="outp", bufs=1))
        rp = ctx.enter_context(tc.tile_pool(name="rp", bufs=8))
        qtcp = ctx.enter_context(tc.tile_pool(name="qtc", bufs=1))

        # Weights: 8 k-tiles each of [128, 256], k-major; wq/wk issue on the
        # DVE queue, wv + biases on gpsimd, first qt chunk on sync so the
        # k=0 projection matmuls start early (ACT issues nothing: it is the
        # exp engine in phase 2)
        wq_t = [
            wts.tile([128, J], BF16, name=f"wq{k}", tag=f"wq{k}") for k in range(NKT)
        ]
        wk_t = [
            wts.tile([128, J], BF16, name=f"wk{k}", tag=f"wk{k}") for k in range(NKT)
        ]
        wv_t = [
            wts.tile([128, J], BF16, name=f"wv{k}", tag=f"wv{k}") for k in range(NKT)
        ]
        qtcs = [
            [
                qtcp.tile([128, 512], BF16, name=f"qtc{c}_{k}", tag=f"qtc{c}_{k}")
                for k in range(NKT)
            ]
            for c in range(NSC)
        ]
        for k in range(NKT):
            ksl = slice(k * 128, (k + 1) * 128)
            nc.sync.dma_start(qtcs[0][k][:], qt[ksl, 0:512])
            nc.scalar.dma_start(wq_t[k][:], wq[ksl, :])
            nc.scalar.dma_start(wk_t[k][:], wk[ksl, :])
            nc.gpsimd.dma_start(wv_t[k][:], wv[ksl, :])
            if k == 3:
                # chunk-1 odd k-tiles are needed ~8us in; don't let them
                # queue behind the remaining weight DMAs
                for kk in range(1, NKT, 2):
                    nc.scalar.dma_start(
                        qtcs[1][kk][:], qt[kk * 128 : (kk + 1) * 128, 512:1024]
                    )
        # prefetch the remaining qt chunks up front so phase 1 never waits
        for c in range(1, NSC):
            for k in range(NKT):
                if c == 1 and k % 2 == 1:
                    continue
                eng = nc.sync if k % 2 == 0 else nc.scalar
                eng.dma_start(
                    qtcs[c][k][:], qt[k * 128 : (k + 1) * 128, c * 512 : c * 512 + 512]
                )

        # Biases: bq/bk as per-partition scalars [128, 2]; bv broadcast [128, 256]
        bq_t = bp.tile([128, 2], F32, name="bqt")
        nc.gpsimd.dma_start(bq_t[:], bq.rearrange("(m p) -> p m", p=128))
        bk_t = bp.tile([128, 2], F32, name="bkt")
        nc.gpsimd.dma_start(bk_t[:], bk.rearrange("(m p) -> p m", p=128))
        bv_t = bp.tile([128, J], F32, name="bvt")
        bvap = bv[:]
        bv_bcast = bass.AP(
            tensor=bvap.tensor, offset=bvap.offset, ap=[[0, 128], [1, J]]
        )
        nc.gpsimd.dma_start(bv_t[:], bv_bcast)

        ident = bp.tile([128, 128], F32, name="ident")
        make_identity(nc, ident[:])
        ident_b = bp.tile([66, 66], BF16, name="identb")
        make_identity(nc, ident_b[:])
        scratch = bp.tile([128, 1], F32, name="scratch")

        # Persistent projected tensors
        qT = [qkp.tile([128, S], BF16, name=f"qT{m}", tag=f"qT{m}") for m in range(2)]
        kT = [qkp.tile([128, S], BF16, name=f"kT{m}", tag=f"kT{m}") for m in range(2)]
        v_ext = []
        for t in range(NTT):
            vt = vxp.tile([128, 4, 65], BF16, name=f"vx{t}", tag=f"vx{t}")
            nc.gpsimd.memset(vt[:], 1.0)  # ones col [:, h, 64] survives
            v_ext.append(vt)
        bv_r = bv_t[:].rearrange("p (h d) -> p h d", h=4)
        # out accumulation: one tile, axis 1 = the 16 128-row output blocks
        out_all = outp.tile([128, 16, J], F32, name="out_all", tag="oall")

        # ---- Phase 1: kT[0], qT[0] chunk 0, and all of v ----
        phase1_qtc = []
        with tc.tile_pool(name="pps", bufs=1, space="PSUM") as pps:
            for sc in range(NSC):
                s0 = sc * 512
                qtc = qtcs[sc]
                # qT[0]/kT[0] are only needed chunk-by-chunk as the pair-0
                # attention blocks consume them, so chunks 1-3 move to
                # deadline-scheduled attention filler
                pq = pps.tile([128, 512], F32, name="pq", tag="pq") if sc == 0 else None
                pk = pps.tile([128, 512], F32, name="pk", tag="pk")
                pv = [
                    pps.tile([128, J], F32, name=f"pv{i}", tag=f"pv{i}")
                    for i in range(4)
                ]
                for k in range(NKT):
                    st, sp = (k == 0), (k == NKT - 1)
                    if pq is not None:
                        nc.tensor.matmul(
                            pq[:], wq_t[k][:, 0:128], qtc[k][:], start=st, stop=sp
                        )
                    nc.tensor.matmul(
                        pk[:], wk_t[k][:, 0:128], qtc[k][:], start=st, stop=sp
                    )
                    for i in range(4):
                        nc.tensor.matmul(
                            pv[i][:],
                            qtc[k][:, i * 128 : (i + 1) * 128],
                            wv_t[k][:],
                            start=st,
                            stop=sp,
                        )
                if pq is not None:
                    nc.scalar.activation(
                        qT[0][:, s0 : s0 + 512], pq[:], IDEN, bias=bq_t[:, 0:1]
                    )
                nc.scalar.activation(
                    kT[0][:, s0 : s0 + 512], pk[:], IDEN, bias=bk_t[:, 0:1]
                )
                phase1_qtc.append(qtc)
                for i in range(4):
                    # fold bv into v (exact through the sum-normalization)
                    nc.vector.tensor_add(
                        v_ext[sc * 4 + i][:, :, 0:64],
                        pv[i][:].rearrange("p (h d) -> p h d", h=4),
                        bv_r,
                    )
                if sc == 0:
                    # pre-load the ACT exp table set during projections so the
                    # first attention exp doesn't stall the pipeline ~2.7us
                    nc.scalar.activation(scratch[:], bq_t[:, 0:1], EXP, scale=0.0)

        # ---- Phase 2: attention, with pair-1 projections and the
        #      transpose/normalize pipeline as PE filler work ----
        with (
            tc.tile_pool(name="aps", bufs=1, space="PSUM") as aps,
            tc.tile_pool(name="p1b", bufs=1, space="PSUM") as p1b,
        ):
            # --- filler: qT[0]/kT[0] chunks 1-3 (read the still-resident
            #     phase-1 qtc tiles; kT chunks carry tight deadlines since
            #     block (0,c) scores at step 4c need kT chunk c) ---
            q0_state = {}

            def uq0_start(key, c):
                def f():
                    q0_state[key] = p1b.tile(
                        [128, 512], F32, name="pq0f", tag=f"x{c % 2}"
                    )
                return f

            def uq0_k(key, c, k, wt, lo):
                def f():
                    st, sp = (k == 0), (k == NKT - 1)
                    nc.tensor.matmul(
                        q0_state[key][:],
                        wt[k][:, lo : lo + 128],
                        phase1_qtc[c][k][:],
                        start=st,
                        stop=sp,
                    )
                return f

            def uq0_copy(key, c, dstT, bias):
                def f():
                    s0 = c * 512
                    nc.scalar.activation(
                        dstT[:, s0 : s0 + 512], q0_state.pop(key)[:],
                        IDEN, bias=bias,
                    )
                return f

            # --- filler: pair-1 projection work units ---
            p1_state = {}

            def u_alloc(c):
                def f():
                    px0 = p1b.tile([128, 512], F32, name="px0", tag="x0")
                    px1 = p1b.tile([128, 512], F32, name="px1", tag="x1")
                    p1_state[c] = (phase1_qtc[c], px0, px1)
                return f

            def u_k(c, k):
                def f():
                    qtc2, px0, px1 = p1_state[c]
                    st, sp = (k == 0), (k == NKT - 1)
                    nc.tensor.matmul(
                        px0[:], wq_t[k][:, 128:256], qtc2[k][:], start=st, stop=sp
                    )
                    nc.tensor.matmul(
                        px1[:], wk_t[k][:, 128:256], qtc2[k][:], start=st, stop=sp
                    )
                return f

            def u_copy(c):
                def f():
                    _, px0, px1 = p1_state.pop(c)
                    s0 = c * 512
                    nc.scalar.activation(
                        qT[1][:, s0 : s0 + 512], px0[:], IDEN, bias=bq_t[:, 1:2]
                    )
                    nc.scalar.activation(
                        kT[1][:, s0 : s0 + 512], px1[:], IDEN, bias=bk_t[:, 1:2]
                    )
                return f

            work = deque()  # entries: (deadline_slot, fn)
            for c in range(1, NSC):
                base = 16 * c - 9
                work.append((base, uq0_start(("q", c), c)))
                for j in range(0, NKT, 2):
                    work.append((base + 1 + j // 2, uq0_k(("q", c), c, j, wq_t, 0)))
                    work.append(
                        (base + 1 + j // 2, uq0_k(("q", c), c, j + 1, wq_t, 0))
                    )
                work.append((16 * c - 4, uq0_copy(("q", c), c, qT[0], bq_t[:, 0:1])))
            for c in range(NSC):
                dl = 44 + 4 * c
                work.append((dl, u_alloc(c)))
                for k in range(NKT):
                    work.append((dl, u_k(c, k)))
                work.append((dl, u_copy(c)))

            # --- filler: transpose/normalize pieces ---
            # per cs tile (one head x one s-chunk): 4 transposes into one
            # PSUM tile (PE queue), then one strided reciprocal for the 4
            # denominators and ONE broadcast tensor_tensor normalize into
            # out_all (DVE queue; popped only on ACT-exp steps so it never
            # delays a DVE exp).
            pe_pieces = deque()
            dve_pieces = deque()
            done_cnt = {}
            piece_idx = [0]
            piece_tags = [("x0", "x1")]

            def grp_alloc(state):
                def f():
                    tags = piece_tags[0]
                    tag = tags[piece_idx[0] % len(tags)]
                    piece_idx[0] += 1
                    if tag.startswith("x"):
                        state["tp"] = p1b.tile([128, 4, 66], BF16, name="tp", tag=tag)
                    else:
                        state["tp"] = aps.tile(
                            [128, 4, 66], BF16, name="tp", tag=tag, bufs=2
                        )
                return f

            def grp_transpose(state, cs_tile, i):
                def f():
                    nc.tensor.transpose(
                        state["tp"][:, i, 0:65],
                        cs_tile[0:65, i * 128 : (i + 1) * 128],
                        ident_b[0:65, 0:65],
                    )
                    if i == 3:
                        state["ready"] = True
                return f

            def grp_recip(state):
                def f():
                    r4 = rp.tile([128, 4, 1], F32, name="r4", tag="r")
                    nc.vector.reciprocal(r4[:], state["tp"][:, :, 64:65])
                    state["r4"] = r4
                return f

            def grp_norm(state, sc, h):
                def f():
                    tp, r4 = state["tp"], state["r4"]
                    rap = r4[:]
                    r_b = bass.AP(
                        tensor=rap.tensor,
                        offset=rap.offset,
                        ap=[rap.ap[0], [rap.ap[1][0], 4], [0, 64]],
                    )
                    nc.vector.tensor_tensor(
                        out_all[:, sc * 4 : sc * 4 + 4, h * 64 : (h + 1) * 64],
                        tp[:, :, 0:64],
                        r_b,
                        mybir.AluOpType.mult,
                    )
                    done_cnt[sc] = done_cnt.get(sc, 0) + 1
                    if done_cnt[sc] == 4:
                        engs = [nc.sync, nc.scalar, nc.gpsimd, nc.sync]
                        for j in range(4):
                            blk = sc * 4 + j
                            engs[j].dma_start(
                                out[blk * 128 : (blk + 1) * 128, :],
                                out_all[:, blk, :],
                            )
                return f

            def add_pieces(cs_tile, sc, h):
                st = {}
                pe_pieces.append(grp_alloc(st))
                for i in range(4):
                    pe_pieces.append(grp_transpose(st, cs_tile, i))
                dve_pieces.append((st, grp_recip(st)))
                dve_pieces.append((st, grp_norm(st, sc, h)))

            slot_no = [0]

            def fill_slot(dve_ok=True):
                # pair-1 projections first (they gate the pair-1 attention
                # blocks at global step 64), then transpose pieces, which
                # reuse the x0/x1 PSUM banks after the projections retire.
                si = slot_no[0]
                slot_no[0] = si + 1
                if work:
                    work.popleft()[1]()
                    while work and work[0][0] <= si + 2:
                        work.popleft()[1]()
                    if work and len(work) > 64 - si:
                        work.popleft()[1]()
                else:
                    eager = si >= 122
                    n = 0
                    while pe_pieces and n < (4 if eager else 2):
                        pe_pieces.popleft()()
                        n += 1
                    if dve_ok or eager:
                        n = 1 + (1 if len(dve_pieces) > 8 else 0)
                        if eager:
                            n = 4
                        while (
                            dve_pieces and n > 0 and dve_pieces[0][0].get("ready")
                        ):
                            dve_pieces.popleft()[1]()
                            n -= 1

            # burst: pair-1 chunk 0 bridges the PSUM pool-transition wait so
            # the PE never idles across the phase boundary (HAM)
            for _ in range(10):
                if work:
                    work.popleft()[1]()

            # One continuous software pipeline across all 8 (pair, s-chunk)
            # blocks: scores/exp for global step i overlap the AV matmuls of
            # step i-1 even across block boundaries, so the exp stream never
            # sees the per-block drain bubble (~1us x 7 otherwise).
            blocks = [(p, sc) for p in range(2) for sc in range(NSC)]
            NB = len(blocks)
            ctx_ps = {}
            pts = {}
            for i in range(NB * NTT + 1):
                if i < NB * NTT:
                    b, t = divmod(i, NTT)
                    pair, sc = blocks[b]
                    s0 = sc * 512
                    qTt, kTt = qT[pair], kT[pair]
                    if t == 0:
                        ctxA = aps.tile(
                            [65, 512], F32, name="ctxA", tag="ctx", bufs=2
                        )
                        ctxB = aps.tile(
                            [65, 512], F32, name="ctxB", tag="ctx", bufs=2
                        )
                        ctx_ps[b] = (ctxA, ctxB)
                    tsl = slice(t * 128, (t + 1) * 128)
                    # both heads' scoresT share one 2-bank PSUM tile; the two
                    # row-group matmuls run concurrently (PSUM-write-bus
                    # capped at ~491ns for the pair)
                    g = aps.tile([128, 1024], F32, name="g", tag="grp", bufs=2)
                    nc.tensor.matmul(
                        g[:, 0:512],
                        kTt[0:64, tsl],
                        qTt[0:64, s0 : s0 + 512],
                        start=True,
                        stop=True,
                        tile_position=(0, 0),
                    )
                    nc.tensor.matmul(
                        g[:, 512:1024],
                        kTt[64:128, tsl],
                        qTt[64:128, s0 : s0 + 512],
                        start=True,
                        stop=True,
                        tile_position=(64, 0),
                    )
                    if i in DVE_EXP:
                        pT_ = pTp.tile([128, 1024], I16, name="pTi", tag="pT")
                        nc.vector.tensor_scalar(
                            pT_[:], g[:], SCH_A, SCH_B,
                            mybir.AluOpType.mult, mybir.AluOpType.add,
                        )
                    else:
                        pT_ = pTp.tile([128, 1024], BF16, name="pTb", tag="pT")
                        nc.scalar.activation(pT_[:], g[:], EXP, scale=0.125)
                    pts[i] = pT_
                if i >= 1:
                    b, t = divmod(i - 1, NTT)
                    pair, sc = blocks[b]
                    hA, hB = 2 * pair, 2 * pair + 1
                    ctxA, ctxB = ctx_ps[b]
                    pT_ = pts.pop(i - 1)
                    if pT_.dtype == I16:
                        mvA = pT_[:, 0:512].bitcast(BF16)
                        mvB = pT_[:, 512:1024].bitcast(BF16)
                    else:
                        mvA = pT_[:, 0:512]
                        mvB = pT_[:, 512:1024]
                    st, sp = (t == 0), (t == NTT - 1)
                    nc.tensor.matmul(
                        ctxA[:], v_ext[t][:, hA, :], mvA, start=st, stop=sp,
                    )
                    nc.tensor.matmul(
                        ctxB[:], v_ext[t][:, hB, :], mvB, start=st, stop=sp,
                    )
                    if t == NTT - 1:
                        del ctx_ps[b]
                        csA = cxp.tile([65, 512], BF16, name="csA", tag="cs")
                        nc.vector.tensor_copy(csA[:], ctxA[:])
                        csB = cxp.tile([65, 512], BF16, name="csB", tag="cs")
                        nc.vector.tensor_copy(csB[:], ctxB[:])
                        add_pieces(csA, sc, hA)
                        add_pieces(csB, sc, hB)
                fill_slot(dve_ok=(i + 1) not in DVE_EXP)

            # drain remaining filler work; the score PSUM banks are free
            # now, so widen the transpose rotation to 4 slots
            while work:
                work.popleft()[1]()
            piece_tags[0] = ("x0", "x1", "grp", "grp")
            while pe_pieces or dve_pieces:
                n = 0
                while pe_pieces and n < 2:
                    pe_pieces.popleft()()
                    n += 1
                while dve_pieces and dve_pieces[0][0].get("ready"):
                    dve_pieces.popleft()[1]()

    nc.compile()
    return nc


def kernel(Q, Wq, bq, Wk, bk, Wv, bv):
    global _cached_nc, last_result
    Q = np.asarray(Q, dtype=np.float32)
    Wq, Wk, Wv = (np.asarray(w, dtype=np.float32) for w in (Wq, Wk, Wv))
    bq, bk, bv = (np.asarray(b, dtype=np.float32) for b in (bq, bk, bv))
    B = Q.shape[0]
    assert Q.shape == (B, S, D) and B * 4 == N_CORES

    if _cached_nc is None:
        _cached_nc = _build()
    nc = _cached_nc

    # host-side shard prep
    qts = [np.ascontiguousarray(Q[b].T).astype(ml_dtypes.bfloat16) for b in range(B)]
    wqs = [
        np.ascontiguousarray(Wq[g * J : (g + 1) * J, :].T).astype(ml_dtypes.bfloat16)
        for g in range(4)
    ]
    wks = [
        np.ascontiguousarray(Wk[g * J : (g + 1) * J, :].T).astype(ml_dtypes.bfloat16)
        for g in range(4)
    ]
    wvs = [
        np.ascontiguousarray(Wv[g * J : (g + 1) * J, :].T).astype(ml_dtypes.bfloat16)
        for g in range(4)
    ]

    in_maps = []
    for c in range(N_CORES):
        b, g = c // 4, c % 4
        jsl = slice(g * J, (g + 1) * J)
        in_maps.append(
            {
                "qt": qts[b],
                "wq": wqs[g],
                "wk": wks[g],
                "wv": wvs[g],
                "bq": np.ascontiguousarray(bq[jsl]),
                "bk": np.ascontiguousarray(bk[jsl]),
                "bv": np.ascontiguousarray(bv[jsl]),
            }
        )

    last_result = run_bass_kernel_spmd(nc, in_maps, list(range(N_CORES)))

    full = np.empty((B, S, D), dtype=np.float32)
    for c in range(N_CORES):
        b, g = c // 4, c % 4
        full[b, :, g * J : (g + 1) * J] = last_result.results[c]["out"]
    return full


# revision 16
# speedup vs baseline: 1.1089x; 1.1089x over previous
"""Multi-head self-attention Trainium2 kernel (8 NeuronCores, SPMD).

Problem: B=2, S=2048, D=1024, H=16, Dk=64; torch-style Linear projections
(x @ W.T + b), custom softmax: p = exp(scores/8), attn = p / (sum(p) + 1e-8).

Sharding: 32 (batch, head) pairs over 8 cores -> core c handles batch c//4,
heads [4*(c%4), 4*(c%4)+4). Each core projects only its 256 features of
q/k/v; attention is embarrassingly parallel over (b, h).

v2 changes over the 232us baseline (trace-driven):
  - The kernel was PE+ACT co-bound (tensor_engine_active 202us, ACT exp
    143us).  PSUM write bus is 1 fp32/cycle/partition: the row-grouped
    score pair is bus-capped at ~491ns regardless of dtype, so scores
    stay fp32r.  The AV path and v-projection move to bf16 (1 cyc/col
    vs fp32r's 1.33): pT (exp output) is bf16, v_ext is bf16 with bv
    pre-folded (exact: sum_t p*bv = denom*bv), wv is bf16.
  - exp is split between ACT (exact, 1.12us/tile) and DVE via a
    Schraudolph bit-trick (i16 = round(score*A + B) bitcast to bf16,
    one tensor_scalar, ~1.24us/tile, +-3.5% on p; end-to-end sim err
    1.0e-2 at 50% offload vs the 2e-2 gate).  Pattern: i%8 in {2,5,7}.
    The two exp engines also overlap adjacent pipeline steps.
  - ACT takes the qk bias-adds (activation Identity, bias is in the
    exp table set: no reload); its DMA issues move to sync/gpsimd/DVE
    queues.  DVE keeps v-fold, ctx copies, reciprocals, normalize.
  - transpose/normalize: 4 transposes of one cs go into one PSUM tile
    so a single strided reciprocal covers all 4 denominators.

Scheduling: unchanged 128-step software pipeline with deficit-paced
filler (qT[0] chunks 1-3, pair-1 projections, transpose/normalize).

Output per core: [2048, 256] fp32 -> host concatenates features per batch.
"""

import sys

sys.path.insert(0, "/opt/trn_rl_repo")

from collections import deque
from contextlib import ExitStack

import numpy as np
import ml_dtypes

import concourse.bass as bass
import concourse.tile as tile
from concourse import bacc, mybir
from concourse.bass_utils import run_bass_kernel_spmd
from concourse.masks import make_identity

F32 = mybir.dt.float32
F32R = mybir.dt.float32r
BF16 = mybir.dt.bfloat16
I16 = mybir.dt.int16

S = 2048  # sequence length
D = 1024  # d_model
J = 256  # features per core (4 heads x 64)
NKT = 8  # k-tiles of the d_model contraction
NSC = 4  # s-chunks of 512
NTT = 16  # t-tiles of 128
N_CORES = 8

# Schraudolph exp2 constants in the bf16-bit domain (exp(x/8) ~ bf16 bits of
# round(x * SCH_A + SCH_B)); c=6 centers the +-3% relative error.
SCH_A = float(128 * np.log2(np.e) / 8)
SCH_B = float(127 * 128 - 6)

# which of the 128 pipeline steps compute exp on the DVE instead of ACT
# (chosen away from block boundaries so a DVE exp never queues behind the
# block-end copy/normalize burst)
DVE_EXP = frozenset(i for i in range(128) if i % 8 in (2, 5, 7))

_cached_nc = None
last_result = None  # BassKernelResults of the most recent run (for test.py)


def _round_fp32r(x: np.ndarray) -> np.ndarray:
    """Round fp32 to fp32r (keep 11 mantissa bits, round to nearest even)."""
    u = np.ascontiguousarray(x, dtype=np.float32).view(np.uint32)
    r = (u.astype(np.uint64) + 0x7FF + ((u >> 12) & 1)) & 0xFFFFF000
    return r.astype(np.uint32).view(np.float32)


def _build():
    nc = bacc.Bacc(None, target_bir_lowering=False)

    qt = nc.dram_tensor("qt", [D, S], BF16, kind="ExternalInput")
    wq = nc.dram_tensor("wq", [D, J], BF16, kind="ExternalInput")
    wk = nc.dram_tensor("wk", [D, J], BF16, kind="ExternalInput")
    wv = nc.dram_tensor("wv", [D, J], BF16, kind="ExternalInput")
    bq = nc.dram_tensor("bq", [J], F32, kind="ExternalInput")
    bk = nc.dram_tensor("bk", [J], F32, kind="ExternalInput")
    bv = nc.dram_tensor("bv", [J], F32, kind="ExternalInput")
    out = nc.dram_tensor("out", [S, J], F32, kind="ExternalOutput")

    IDEN = mybir.ActivationFunctionType.Identity
    EXP = mybir.ActivationFunctionType.Exp

    with tile.TileContext(nc) as tc, ExitStack() as ctx:
        wts = ctx.enter_context(tc.tile_pool(name="wts", bufs=1))
        qkp = ctx.enter_context(tc.tile_pool(name="qkp", bufs=1))
        vxp = ctx.enter_context(tc.tile_pool(name="vxp", bufs=1))
        bp = ctx.enter_context(tc.tile_pool(name="bp", bufs=1))
        cxp = ctx.enter_context(tc.tile_pool(name="cxp", bufs=6))
        pTp = ctx.enter_context(tc.tile_pool(name="pTp", bufs=4))
        outp = ctx.enter_context(tc.tile_pool(name="outp", bufs=1))
        rp = ctx.enter_context(tc.tile_pool(name="rp", bufs=8))
        qtcp = ctx.enter_context(tc.tile_pool(name="qtc", bufs=1))

        # Weights: 8 k-tiles each of [128, 256], k-major; wq/wk issue on the
        # DVE queue, wv + biases on gpsimd, first qt chunk on sync so the
        # k=0 projection matmuls start early (ACT issues nothing: it is the
        # exp engine in phase 2)
        wq_t = [
            wts.tile([128, J], BF16, name=f"wq{k}", tag=f"wq{k}") for k in range(NKT)
        ]
        wk_t = [
            wts.tile([128, J], BF16, name=f"wk{k}", tag=f"wk{k}") for k in range(NKT)
        ]
        wv_t = [
            wts.tile([128, J], BF16, name=f"wv{k}", tag=f"wv{k}") for k in range(NKT)
        ]
        qtcs = [
            [
                qtcp.tile([128, 512], BF16, name=f"qtc{c}_{k}", tag=f"qtc{c}_{k}")
                for k in range(NKT)
            ]
            for c in range(NSC)
        ]
        for k in range(NKT):
            ksl = slice(k * 128, (k + 1) * 128)
            nc.sync.dma_start(qtcs[0][k][:], qt[ksl, 0:512])
            nc.scalar.dma_start(wq_t[k][:], wq[ksl, :])
            nc.scalar.dma_start(wk_t[k][:], wk[ksl, :])
            nc.gpsimd.dma_start(wv_t[k][:], wv[ksl, :])
            if k == 3:
                # chunk-1 odd k-tiles are needed ~8us in; don't let them
                # queue behind the remaining weight DMAs
                for kk in range(1, NKT, 2):
                    nc.scalar.dma_start(
                        qtcs[1][kk][:], qt[kk * 128 : (kk + 1) * 128, 512:1024]
                    )
        # prefetch the remaining qt chunks up front so phase 1 never waits
        for c in range(1, NSC):
            for k in range(NKT):
                if c == 1 and k % 2 == 1:
                    continue
                eng = nc.sync if k % 2 == 0 else nc.scalar
                eng.dma_start(
                    qtcs[c][k][:], qt[k * 128 : (k + 1) * 128, c * 512 : c * 512 + 512]
                )

        # Biases: bq/bk as per-partition scalars [128, 2]; bv broadcast [128, 256]
        bq_t = bp.tile([128, 2], F32, name="bqt")
        nc.gpsimd.dma_start(bq_t[:], bq.rearrange("(m p) -> p m", p=128))
        bk_t = bp.tile([128, 2], F32, name="bkt")
        nc.gpsimd.dma_start(bk_t[:], bk.rearrange("(m p) -> p m", p=128))
        bv_t = bp.tile([128, J], F32, name="bvt")
        bvap = bv[:]
        bv_bcast = bass.AP(
            tensor=bvap.tensor, offset=bvap.offset, ap=[[0, 128], [1, J]]
        )
        nc.gpsimd.dma_start(bv_t[:], bv_bcast)

        ident = bp.tile([128, 128], F32, name="ident")
        make_identity(nc, ident[:])
        ident_b = bp.tile([66, 66], BF16, name="identb")
        make_identity(nc, ident_b[:])
        scratch = bp.tile([128, 1], F32, name="scratch")

        # Persistent projected tensors
        qT = [qkp.tile([128, S], F32R, name=f"qT{m}", tag=f"qT{m}") for m in range(2)]
        kT = [qkp.tile([128, S], F32R, name=f"kT{m}", tag=f"kT{m}") for m in range(2)]
        v_ext = []
        for t in range(NTT):
            vt = vxp.tile([128, 4, 65], BF16, name=f"vx{t}", tag=f"vx{t}")
            nc.gpsimd.memset(vt[:], 1.0)  # ones col [:, h, 64] survives
            v_ext.append(vt)
        bv_r = bv_t[:].rearrange("p (h d) -> p h d", h=4)
        # out accumulation: one tile, axis 1 = the 16 128-row output blocks
        out_all = outp.tile([128, 16, J], F32, name="out_all", tag="oall")

        # ---- Phase 1: kT[0], qT[0] chunk 0, and all of v ----
        phase1_qtc = []
        with tc.tile_pool(name="pps", bufs=1, space="PSUM") as pps:
            for sc in range(NSC):
                s0 = sc * 512
                qtc = qtcs[sc]
                # qT[0]/kT[0] are only needed chunk-by-chunk as the pair-0
                # attention blocks consume them, so chunks 1-3 move to
                # deadline-scheduled attention filler
                pq = pps.tile([128, 512], F32, name="pq", tag="pq") if sc == 0 else None
                pk = pps.tile([128, 512], F32, name="pk", tag="pk")
                pv = [
                    pps.tile([128, J], F32, name=f"pv{i}", tag=f"pv{i}")
                    for i in range(4)
                ]
                for k in range(NKT):
                    st, sp = (k == 0), (k == NKT - 1)
                    if pq is not None:
                        nc.tensor.matmul(
                            pq[:], wq_t[k][:, 0:128], qtc[k][:], start=st, stop=sp
                        )
                    nc.tensor.matmul(
                        pk[:], wk_t[k][:, 0:128], qtc[k][:], start=st, stop=sp
                    )
                    for i in range(4):
                        nc.tensor.matmul(
                            pv[i][:],
                            qtc[k][:, i * 128 : (i + 1) * 128],
                            wv_t[k][:],
                            start=st,
                            stop=sp,
                        )
                if pq is not None:
                    nc.scalar.activation(
                        qT[0][:, s0 : s0 + 512], pq[:], IDEN, bias=bq_t[:, 0:1]
                    )
                nc.scalar.activation(
                    kT[0][:, s0 : s0 + 512], pk[:], IDEN, bias=bk_t[:, 0:1]
                )
                phase1_qtc.append(qtc)
                for i in range(4):
                    # fold bv into v (exact through the sum-normalization)
                    nc.vector.tensor_add(
                        v_ext[sc * 4 + i][:, :, 0:64],
                        pv[i][:].rearrange("p (h d) -> p h d", h=4),
                        bv_r,
                    )
                if sc == 0:
                    # pre-load the ACT exp table set during projections so the
                    # first attention exp doesn't stall the pipeline ~2.7us
                    nc.scalar.activation(scratch[:], bq_t[:, 0:1], EXP, scale=0.0)

        # ---- Phase 2: attention, with pair-1 projections and the
        #      transpose/normalize pipeline as PE filler work ----
        with (
            tc.tile_pool(name="aps", bufs=1, space="PSUM") as aps,
            tc.tile_pool(name="p1b", bufs=1, space="PSUM") as p1b,
        ):
            # --- filler: qT[0]/kT[0] chunks 1-3 (read the still-resident
            #     phase-1 qtc tiles; kT chunks carry tight deadlines since
            #     block (0,c) scores at step 4c need kT chunk c) ---
            q0_state = {}

            def uq0_start(key, c):
                def f():
                    q0_state[key] = p1b.tile(
                        [128, 512], F32, name="pq0f", tag=f"x{c % 2}"
                    )
                return f

            def uq0_k(key, c, k, wt, lo):
                def f():
                    st, sp = (k == 0), (k == NKT - 1)
                    nc.tensor.matmul(
                        q0_state[key][:],
                        wt[k][:, lo : lo + 128],
                        phase1_qtc[c][k][:],
                        start=st,
                        stop=sp,
                    )
                return f

            def uq0_copy(key, c, dstT, bias):
                def f():
                    s0 = c * 512
                    nc.scalar.activation(
                        dstT[:, s0 : s0 + 512], q0_state.pop(key)[:],
                        IDEN, bias=bias,
                    )
                return f

            # --- filler: pair-1 projection work units ---
            p1_state = {}

            def u_alloc(c):
                def f():
                    px0 = p1b.tile([128, 512], F32, name="px0", tag="x0")
                    px1 = p1b.tile([128, 512], F32, name="px1", tag="x1")
                    p1_state[c] = (phase1_qtc[c], px0, px1)
                return f

            def u_k(c, k):
                def f():
                    qtc2, px0, px1 = p1_state[c]
                    st, sp = (k == 0), (k == NKT - 1)
                    nc.tensor.matmul(
                        px0[:], wq_t[k][:, 128:256], qtc2[k][:], start=st, stop=sp
                    )
                    nc.tensor.matmul(
                        px1[:], wk_t[k][:, 128:256], qtc2[k][:], start=st, stop=sp
                    )
                return f

            def u_copy(c):
                def f():
                    _, px0, px1 = p1_state.pop(c)
                    s0 = c * 512
                    nc.scalar.activation(
                        qT[1][:, s0 : s0 + 512], px0[:], IDEN, bias=bq_t[:, 1:2]
                    )
                    nc.scalar.activation(
                        kT[1][:, s0 : s0 + 512], px1[:], IDEN, bias=bk_t[:, 1:2]
                    )
                return f

            work = deque()  # entries: (deadline_slot, fn)
            for c in range(1, NSC):
                base = 16 * c - 9
                work.append((base, uq0_start(("q", c), c)))
                for j in range(0, NKT, 2):
                    work.append((base + 1 + j // 2, uq0_k(("q", c), c, j, wq_t, 0)))
                    work.append(
                        (base + 1 + j // 2, uq0_k(("q", c), c, j + 1, wq_t, 0))
                    )
                work.append((16 * c - 4, uq0_copy(("q", c), c, qT[0], bq_t[:, 0:1])))
            for c in range(NSC):
                dl = 44 + 4 * c
                work.append((dl, u_alloc(c)))
                for k in range(NKT):
                    work.append((dl, u_k(c, k)))
                work.append((dl, u_copy(c)))

            # --- filler: transpose/normalize pieces ---
            # per cs tile (one head x one s-chunk): 4 transposes into one
            # PSUM tile (PE queue), then one strided reciprocal for the 4
            # denominators and ONE broadcast tensor_tensor normalize into
            # out_all (DVE queue; popped only on ACT-exp steps so it never
            # delays a DVE exp).
            pe_pieces = deque()
            dve_pieces = deque()
            done_cnt = {}
            piece_idx = [0]
            piece_tags = [("x0", "x1")]

            def grp_alloc(state):
                def f():
                    tags = piece_tags[0]
                    tag = tags[piece_idx[0] % len(tags)]
                    piece_idx[0] += 1
                    if tag.startswith("x"):
                        state["tp"] = p1b.tile([128, 4, 66], BF16, name="tp", tag=tag)
                    else:
                        state["tp"] = aps.tile(
                            [128, 4, 66], BF16, name="tp", tag=tag, bufs=2
                        )
                return f

            def grp_transpose(state, cs_tile, i):
                def f():
                    nc.tensor.transpose(
                        state["tp"][:, i, 0:65],
                        cs_tile[0:65, i * 128 : (i + 1) * 128],
                        ident_b[0:65, 0:65],
                    )
                    if i == 3:
                        state["ready"] = True
                return f

            def grp_recip(state):
                def f():
                    r4 = rp.tile([128, 4, 1], F32, name="r4", tag="r")
                    nc.vector.reciprocal(r4[:], state["tp"][:, :, 64:65])
                    state["r4"] = r4
                return f

            def grp_norm(state, sc, h):
                def f():
                    tp, r4 = state["tp"], state["r4"]
                    rap = r4[:]
                    r_b = bass.AP(
                        tensor=rap.tensor,
                        offset=rap.offset,
                        ap=[rap.ap[0], [rap.ap[1][0], 4], [0, 64]],
                    )
                    nc.vector.tensor_tensor(
                        out_all[:, sc * 4 : sc * 4 + 4, h * 64 : (h + 1) * 64],
                        tp[:, :, 0:64],
                        r_b,
                        mybir.AluOpType.mult,
                    )
                    done_cnt[sc] = done_cnt.get(sc, 0) + 1
                    if done_cnt[sc] == 4:
                        engs = [nc.sync, nc.scalar, nc.gpsimd, nc.sync]
                        for j in range(4):
                            blk = sc * 4 + j
                            engs[j].dma_start(
                                out[blk * 128 : (blk + 1) * 128, :],
                                out_all[:, blk, :],
                            )
                return f

            def add_pieces(cs_tile, sc, h):
                st = {}
                pe_pieces.append(grp_alloc(st))
                for i in range(4):
                    pe_pieces.append(grp_transpose(st, cs_tile, i))
                dve_pieces.append((st, grp_recip(st)))
                dve_pieces.append((st, grp_norm(st, sc, h)))

            slot_no = [0]

            def fill_slot(dve_ok=True):
                # pair-1 projections first (they gate the pair-1 attention
                # blocks at global step 64), then transpose pieces, which
                # reuse the x0/x1 PSUM banks after the projections retire.
                si = slot_no[0]
                slot_no[0] = si + 1
                if work:
                    work.popleft()[1]()
                    while work and work[0][0] <= si + 2:
                        work.popleft()[1]()
                    if work and len(work) > 64 - si:
                        work.popleft()[1]()
                else:
                    eager = si >= 122
                    n = 0
                    while pe_pieces and n < (4 if eager else 2):
                        pe_pieces.popleft()()
                        n += 1
                    if dve_ok or eager:
                        n = 1 + (1 if len(dve_pieces) > 8 else 0)
                        if eager:
                            n = 4
                        while (
                            dve_pieces and n > 0 and dve_pieces[0][0].get("ready")
                        ):
                            dve_pieces.popleft()[1]()
                            n -= 1

            # burst: pair-1 chunk 0 bridges the PSUM pool-transition wait so
            # the PE never idles across the phase boundary (HAM)
            for _ in range(10):
                if work:
                    work.popleft()[1]()

            # One continuous software pipeline across all 8 (pair, s-chunk)
            # blocks: scores/exp for global step i overlap the AV matmuls of
            # step i-1 even across block boundaries, so the exp stream never
            # sees the per-block drain bubble (~1us x 7 otherwise).
            blocks = [(p, sc) for p in range(2) for sc in range(NSC)]
            NB = len(blocks)
            ctx_ps = {}
            pts = {}
            for i in range(NB * NTT + 1):
                if i < NB * NTT:
                    b, t = divmod(i, NTT)
                    pair, sc = blocks[b]
                    s0 = sc * 512
                    qTt, kTt = qT[pair], kT[pair]
                    if t == 0:
                        ctxA = aps.tile(
                            [65, 512], F32, name="ctxA", tag="ctx", bufs=2
                        )
                        ctxB = aps.tile(
                            [65, 512], F32, name="ctxB", tag="ctx", bufs=2
                        )
                        ctx_ps[b] = (ctxA, ctxB)
                    tsl = slice(t * 128, (t + 1) * 128)
                    # both heads' scoresT share one 2-bank PSUM tile; the two
                    # row-group matmuls run concurrently (PSUM-write-bus
                    # capped at ~491ns for the pair)
                    g = aps.tile([128, 1024], F32, name="g", tag="grp", bufs=2)
                    nc.tensor.matmul(
                        g[:, 0:512],
                        kTt[0:64, tsl],
                        qTt[0:64, s0 : s0 + 512],
                        start=True,
                        stop=True,
                        tile_position=(0, 0),
                    )
                    nc.tensor.matmul(
                        g[:, 512:1024],
                        kTt[64:128, tsl],
                        qTt[64:128, s0 : s0 + 512],
                        start=True,
                        stop=True,
                        tile_position=(64, 0),
                    )
                    if i in DVE_EXP:
                        pT_ = pTp.tile([128, 1024], I16, name="pTi", tag="pT")
                        nc.vector.tensor_scalar(
                            pT_[:], g[:], SCH_A, SCH_B,
                            mybir.AluOpType.mult, mybir.AluOpType.add,
                        )
                    else:
                        pT_ = pTp.tile([128, 1024], BF16, name="pTb", tag="pT")
                        nc.scalar.activation(pT_[:], g[:], EXP, scale=0.125)
                    pts[i] = pT_
                if i >= 1:
                    b, t = divmod(i - 1, NTT)
                    pair, sc = blocks[b]
                    hA, hB = 2 * pair, 2 * pair + 1
                    ctxA, ctxB = ctx_ps[b]
                    pT_ = pts.pop(i - 1)
                    if pT_.dtype == I16:
                        mvA = pT_[:, 0:512].bitcast(BF16)
                        mvB = pT_[:, 512:1024].bitcast(BF16)
                    else:
                        mvA = pT_[:, 0:512]
                        mvB = pT_[:, 512:1024]
                    st, sp = (t == 0), (t == NTT - 1)
                    nc.tensor.matmul(
                        ctxA[:], v_ext[t][:, hA, :], mvA, start=st, stop=sp,
                    )
                    nc.tensor.matmul(
                        ctxB[:], v_ext[t][:, hB, :], mvB, start=st, stop=sp,
                    )
                    if t == NTT - 1:
                        del ctx_ps[b]
                        csA = cxp.tile([65, 512], BF16, name="csA", tag="cs")
                        nc.vector.tensor_copy(csA[:], ctxA[:])
                        csB = cxp.tile([65, 512], BF16, name="csB", tag="cs")
                        nc.vector.tensor_copy(csB[:], ctxB[:])
                        add_pieces(csA, sc, hA)
                        add_pieces(csB, sc, hB)
                fill_slot(dve_ok=(i + 1) not in DVE_EXP)

            # drain remaining filler work; the score PSUM banks are free
            # now, so widen the transpose rotation to 4 slots
            while work:
                work.popleft()[1]()
            piece_tags[0] = ("x0", "x1", "grp", "grp")
            while pe_pieces or dve_pieces:
                n = 0
                while pe_pieces and n < 2:
                    pe_pieces.popleft()()
                    n += 1
                while dve_pieces and dve_pieces[0][0].get("ready"):
                    dve_pieces.popleft()[1]()

    nc.compile()
    return nc


def kernel(Q, Wq, bq, Wk, bk, Wv, bv):
    global _cached_nc, last_result
    Q = np.asarray(Q, dtype=np.float32)
    Wq, Wk, Wv = (np.asarray(w, dtype=np.float32) for w in (Wq, Wk, Wv))
    bq, bk, bv = (np.asarray(b, dtype=np.float32) for b in (bq, bk, bv))
    B = Q.shape[0]
    assert Q.shape == (B, S, D) and B * 4 == N_CORES

    if _cached_nc is None:
        _cached_nc = _build()
    nc = _cached_nc

    # host-side shard prep
    qts = [np.ascontiguousarray(Q[b].T).astype(ml_dtypes.bfloat16) for b in range(B)]
    wqs = [
        np.ascontiguousarray(Wq[g * J : (g + 1) * J, :].T).astype(ml_dtypes.bfloat16)
        for g in range(4)
    ]
    wks = [
        np.ascontiguousarray(Wk[g * J : (g + 1) * J, :].T).astype(ml_dtypes.bfloat16)
        for g in range(4)
    ]
    wvs = [
        np.ascontiguousarray(Wv[g * J : (g + 1) * J, :].T).astype(ml_dtypes.bfloat16)
        for g in range(4)
    ]

    in_maps = []
    for c in range(N_CORES):
        b, g = c // 4, c % 4
        jsl = slice(g * J, (g + 1) * J)
        in_maps.append(
            {
                "qt": qts[b],
                "wq": wqs[g],
                "wk": wks[g],
                "wv": wvs[g],
                "bq": np.ascontiguousarray(bq[jsl]),
                "bk": np.ascontiguousarray(bk[jsl]),
                "bv": np.ascontiguousarray(bv[jsl]),
            }
        )

    last_result = run_bass_kernel_spmd(nc, in_maps, list(range(N_CORES)))

    full = np.empty((B, S, D), dtype=np.float32)
    for c in range(N_CORES):
        b, g = c // 4, c % 4
        full[b, :, g * J : (g + 1) * J] = last_result.results[c]["out"]
    return full
